# revision 11
# baseline (speedup 1.0000x reference)
"""AffinityFC Trainium2 kernel (Bass/Tile, 8 NeuronCores, data-parallel over B).

Math per batch b (one NeuronCore per batch):
    px = X[b] @ W1x.T          (Nx=128, hd=1024)
    py = Y[b] @ W1y.T          (Ny=128, hd=1024)
    out[n, m] = W2 . relu(px[n, :] + py[m, :] + b1) + b2

Key reformulation: with s = px + b1,
    relu(py + s) = max(py, -s) + s
so "max-form" rows compute u = max(py, -s) (one DVE tensor_tensor max
per element at 2x bf16 rate) and reduce Sum_h W2[h]*u with TensorE;
the missing Sum_h W2[h]*s[n,h] term is a rank-1 correction added on
the host.  "relu-form" rows are computed on ScalarE as
relu(py + s[n]) with a per-partition bias, needing no correction.

v21 schedule:
  - Row split is PURE per row-block: nb 0..26 are max-form on DVE in
    every chunk (one w27 TENSOR_TENSOR max per chunk, (m,j)-interleaved
    rhs layout); nb 27..31 are relu-form on ScalarE in every chunk with
    a DENSE (j,m)-block layout (dense src py_plain, dense dst), fully
    decoupled from the DVE chunk loop: the 160 activations stream right
    behind layer-1, their bank-6/7 matmuls run early.
  - PSUM: obanks 0..5 allocated before the layer-1 pool (so their
    matmuls can start immediately); obanks 6,7 reuse layer-1's 2 banks
    after it closes.  Banks 0..5 hold DVE rows {bk, bk+6, bk+12,
    bk+18}; bank 7 holds rows 24,25,26 (DVE) + 27 (ScalarE, jc3);
    bank 6 holds rows 28..31 (ScalarE).
  - Reduction matmuls are issued jc-interleaved so 4 col-groups of the
    PE array run concurrently.
  - DMA: sync queue carries cry=[yt|w1y_c0] then bulk w1y in 2 groups
    then all 8 output DMAs; scalar queue carries crx=[xt|w1x_c0], the
    small consts, then bulk w1x in 2 groups.  Output is bf16 raw.
  - Final psum evacs are split: ScalarE takes banks 0..2 + 6, DVE
    (idle after its last max op) takes banks 3,4,5,7.
  Known hazard: DVE/PE clocks vary between runs (0.96 vs 0.80 GHz);
  compare designs only at equal clock.
"""

import numpy as np
import ml_dtypes

import concourse.mybir as mybir
import concourse.tile as tile
from concourse import bacc
from concourse.bass import ts
from concourse.bass_utils import run_bass_kernel_spmd

B, NX, NY, D, HD = 8, 128, 128, 512, 1024
NCORES = 8
NCH = HD // 128      # 8 h-chunks
KT = D // 128        # 4 k-tiles for the layer-1 contraction
NBLK = NX // 4       # 32 n-blocks of 4 rows each
F32 = mybir.dt.float32
BF16 = mybir.dt.bfloat16

SNBS = (27, 28, 29, 30, 31)   # relu-form rows (ScalarE), all chunks
NDVE = 27                     # nb 0..26 on DVE


def bankmap(nb):  # nb -> (bank, jc)
    if nb < 24:
        return nb % 6, nb // 6
    if nb < 27:
        return 7, nb - 24
    if nb == 27:
        return 7, 3
    return 6, nb - 28


def bank_rows(bk):
    if bk < 6:
        return [bk, bk + 6, bk + 12, bk + 18]
    if bk == 7:
        return [24, 25, 26, 27]
    return [28, 29, 30, 31]


def _build_nc(do_compile=True):
    nc = bacc.Bacc(
        "TRN2", target_bir_lowering=False, debug=False, num_devices=NCORES
    )

    # flat SBUF images: dram[p, col] == sbuf[p, col]
    # crx/cry bundle the chunk-0-critical data into ONE DMA each:
    #   crx = [xt image (KT*NX) | w1x chunk-0 slab (KT*128)]
    CW0 = KT * 128
    crx = nc.dram_tensor("crx", [128, KT * NX + CW0], BF16, kind="ExternalInput")
    cry = nc.dram_tensor("cry", [128, KT * NY + CW0], BF16, kind="ExternalInput")
    w1xt = nc.dram_tensor("w1xt", [128, (NCH - 1) * CW0], BF16, kind="ExternalInput")
    w1yt = nc.dram_tensor("w1yt", [128, (NCH - 1) * CW0], BF16, kind="ExternalInput")
    b1c = nc.dram_tensor("b1c", [128, 2 * NCH], F32, kind="ExternalInput")
    w2c = nc.dram_tensor("w2c", [128, NCH * 32], BF16, kind="ExternalInput")
    out = nc.dram_tensor("out", [1, NBLK * 512], BF16, kind="ExternalOutput")

    with tile.TileContext(nc) as tc:
        with (
            tc.tile_pool(name="const", bufs=1) as cp,
            tc.tile_pool(name="tprod", bufs=4) as tp,
            tc.tile_pool(name="taprod", bufs=12) as tap,
        ):
            crx_sb = cp.tile([128, KT * NX + CW0], BF16)
            cry_sb = cp.tile([128, KT * NY + CW0], BF16)
            xt_sb = crx_sb[:, : KT * NX]
            yt_sb = cry_sb[:, : KT * NY]
            W1GRP = ((1, 4), (4, NCH))
            w1x_g = [
                cp.tile([128, (hi - lo) * KT * 128], BF16, name=f"w1x{lo}")
                for lo, hi in W1GRP
            ]
            w1y_g = [
                cp.tile([128, (hi - lo) * KT * 128], BF16, name=f"w1y{lo}")
                for lo, hi in W1GRP
            ]

            def w1slab(g, c, k):  # lhsT slab for (chunk c, k-tile)
                if c == 0:
                    cr = crx_sb if g is w1x_g else cry_sb
                    off = KT * NX + k * 128
                    return cr[:, off : off + 128]
                for (lo, hi), tile_ in zip(W1GRP, g):
                    if lo <= c < hi:
                        off = ((c - lo) * KT + k) * 128
                        return tile_[:, off : off + 128]
                raise AssertionError

            dummy_sb = cp.tile([128, 64], BF16)     # PE warmup garbage
            b1_sb = cp.tile([128, 2 * NCH], F32)    # [+b1 | -b1] chunk columns
            w2_sb = cp.tile([128, NCH * 32], BF16)
            negs_sb = cp.tile([128, HD], BF16)      # -(px+b1), bf16
            s20_sb = cp.tile([128, NCH * 20], F32)  # s = px+b1 for rows 108..127
            pyr_sb = cp.tile([128, NCH * 512], BF16)  # py rep4 (m,j) per chunk
            pyp_sb = cp.tile([128, NCH * 128], BF16)  # py plain per chunk
            out_sc = cp.tile([128, 8 * 512], BF16)

            # ---- input DMAs.
            # sync queue: cry first (y side gates the first max), then
            # bulk w1y in two groups.  scalar queue: crx, consts, bulk
            # w1x in two groups.  Outputs go on sync at the end.
            nc.sync.dma_start(out=cry_sb[:, :], in_=cry[:, :])
            nc.scalar.dma_start(out=crx_sb[:, :], in_=crx[:, :])
            nc.scalar.dma_start(out=b1_sb[:, :], in_=b1c[:, :])
            nc.scalar.dma_start(out=w2_sb[:, :], in_=w2c[:, :])
            # bulk W1 rides the sync queue (y/x interleaved by deadline)
            # so the Scalar engine is free for the evac stream
            nc.sync.dma_start(out=w1y_g[0][:, :], in_=w1yt[:, : 3 * CW0])
            nc.sync.dma_start(out=w1x_g[0][:, :], in_=w1xt[:, : 3 * CW0])
            nc.sync.dma_start(out=w1y_g[1][:, :], in_=w1yt[:, 3 * CW0 :])
            nc.sync.dma_start(out=w1x_g[1][:, :], in_=w1xt[:, 3 * CW0 :])

            with tc.tile_pool(name="mpsA", bufs=1, space="PSUM") as mpsA:
                obanks = [None] * 8
                for i in range(6):
                    obanks[i] = mpsA.tile([128, 512], F32, name=f"ob{i}", tag=f"ob{i}")

                # PE warmup: ~30 no-input matmuls flip the HAM clock gate
                # to 8/8 before the first real layer-1 matmuls arrive.
                nc.gpsimd.memset(dummy_sb[:, :], 0.0)
                for _ in range(30):
                    nc.tensor.matmul(
                        obanks[0][0:64, 0:64],
                        dummy_sb[:, :],
                        dummy_sb[:, :],
                        start=True,
                        stop=True,
                        skip_group_check=True,
                    )

                # ---- layer 1 per h-chunk + the full ScalarE act stream.
                # ScalarE FIFO order: [c evacs + acts] for c=0..2, then
                # evacs-only for c=3..7, then c7 acts (so the bank-6/7
                # relu-form accumulation can finish off the tail), then
                # the deferred c3..c6 acts.
                ta_tiles = {}  # (nb, c) -> tile
                deferred_act_cs = []

                def emit_acts(c):
                    for nb in SNBS:
                        ta = tap.tile(
                            [128, 512], BF16, name=f"ta{c}_{nb}", tag="ta"
                        )
                        ta_tiles[(nb, c)] = ta
                        for j in range(4):
                            n = nb * 4 + j
                            nc.scalar.activation(
                                out=ta[:, ts(j, 128)],
                                in_=pyp_sb[:, ts(c, 128)],
                                func=mybir.ActivationFunctionType.Relu,
                                bias=s20_sb[:, c * 20 + n - 108 : c * 20 + n - 107],
                                scale=1.0,
                            )

                with tc.tile_pool(name="l1ps", bufs=2, space="PSUM") as l1ps:
                    for c in range(NCH):
                        pyp = l1ps.tile([128, NY], F32, tag="l1")
                        for k in range(KT):
                            nc.tensor.matmul(
                                pyp[:, :],
                                w1slab(w1y_g, c, k),
                                yt_sb[:, ts(k, NY)],
                                start=(k == 0),
                                stop=(k == KT - 1),
                            )
                        with tc.high_priority():
                            nc.scalar.activation(
                                out=pyr_sb[:, ts(c, 512)].rearrange(
                                    "p (m j) -> p m j", j=4
                                ),
                                in_=pyp[:, :].unsqueeze(2).broadcast_to(
                                    (128, 128, 4)
                                ),
                                func=mybir.ActivationFunctionType.Copy,
                            )
                            nc.scalar.copy(
                                out=pyp_sb[:, ts(c, 128)], in_=pyp[:, :]
                            )
                        pxp = l1ps.tile([128, NX], F32, tag="l1")
                        for k in range(KT):
                            nc.tensor.matmul(
                                pxp[:, :],
                                w1slab(w1x_g, c, k),
                                xt_sb[:, ts(k, NX)],
                                start=(k == 0),
                                stop=(k == KT - 1),
                            )
                        with tc.high_priority():
                            nc.scalar.activation(
                                out=negs_sb[:, ts(c, 128)],
                                in_=pxp[:, :],
                                func=mybir.ActivationFunctionType.Identity,
                                bias=b1_sb[:, NCH + c : NCH + c + 1],
                                scale=-1.0,
                            )
                            nc.scalar.activation(
                                out=s20_sb[:, ts(c, 20)],
                                in_=pxp[:, 108:128],
                                func=mybir.ActivationFunctionType.Identity,
                                bias=b1_sb[:, c : c + 1],
                                scale=1.0,
                            )
                        if c == 0:
                            # bulk w1x DMA issues ride behind the chunk-0
                            # evacs on the Scalar engine
                            nc.scalar.dma_start(
                                out=w1x_g[0][:, :], in_=w1xt[:, : 3 * CW0]
                            )
                            nc.scalar.dma_start(
                                out=w1x_g[1][:, :], in_=w1xt[:, 3 * CW0 :]
                            )
                        if c <= 2:
                            emit_acts(c)
                        elif c < NCH - 1:
                            deferred_act_cs.append(c)
                        else:
                            emit_acts(c)            # c7 acts hoisted
                            for dc in deferred_act_cs:
                                emit_acts(dc)

                # banks 6,7 reuse layer-1's psum space (deps via allocator)
                mpsB_cm = tc.tile_pool(name="mpsB", bufs=1, space="PSUM")
                mpsB = mpsB_cm.__enter__()
                obanks[6] = mpsB.tile([128, 512], F32, name="ob6", tag="ob6")
                obanks[7] = mpsB.tile([128, 512], F32, name="ob7", tag="ob7")

                # jc-interleaved issue order for the 27 DVE rows (4-way
                # col-group concurrency on the PE array)
                dve_order = []
                for r in range(6):
                    for j in range(4):
                        dve_order.append(6 * j + ((r + j) % 6))
                dve_order = [nb for nb in dve_order if nb < 24]
                dve_order += [24, 25, 26]

                # SNBS (relu-form) matmuls are issued one per red-group;
                # chunk 7's acts are hoisted so its matmul can run early,
                # and the stop flag goes on the last-issued chunk (c6 —
                # accumulation order is irrelevant, adds commute).
                SNBS_SCHED = [0, 1, 2, 7, 3, 4, 5, 6]

                # ---- main loop, c-outer
                for c in range(NCH):
                    last = c == NCH - 1
                    pyr_c = pyr_sb[:, ts(c, 512)]
                    pyr3 = pyr_c.rearrange("p (m j) -> p m j", j=4)
                    tslice = {}  # nb -> (tile, column offset index)

                    def dve_max(t, w, in1, prio):
                        in0 = pyr3.unsqueeze(1).broadcast_to((128, w, 128, 4))
                        out_ap = t[:, :].rearrange(
                            "p (nbs m j) -> p nbs m j", nbs=w, m=128
                        )
                        if prio:
                            with tc.high_priority():
                                nc.vector.tensor_tensor(
                                    out=out_ap, in0=in0, in1=in1,
                                    op=mybir.AluOpType.max,
                                )
                        else:
                            nc.vector.tensor_tensor(
                                out=out_ap, in0=in0, in1=in1,
                                op=mybir.AluOpType.max,
                            )

                    def negs_in1(cols, w):
                        # cols: list-slice of negs columns [p, w, 4] -> bcast m
                        return cols.unsqueeze(2).broadcast_to((128, w, 128, 4))

                    if not last:
                        t = tp.tile([128, NDVE * 512], BF16, name=f"t{c}",
                                    tag="t")
                        in1 = negs_in1(
                            negs_sb[:, c * 128 : c * 128 + 4 * NDVE]
                            .rearrange("p (nbs j) -> p nbs j", j=4), NDVE
                        )
                        dve_max(t, NDVE, in1, c == 0)
                        for nbs in range(NDVE):
                            tslice[nbs] = (t, nbs)
                    else:
                        # bank-grouped ops so the final bank-major
                        # matmul+evac+DMA pipeline starts per-bank
                        nrr = negs_sb[:, ts(c, 128)].rearrange(
                            "p (nb j) -> p nb j", j=4
                        )
                        for bk in range(6):
                            t = tp.tile(
                                [128, 4 * 512], BF16, name=f"tb{bk}",
                                tag="t4", bufs=6,
                            )
                            in1 = negs_in1(
                                nrr[:, bk : bk + 19 : 6, :], 4
                            )
                            dve_max(t, 4, in1, False)
                            for i, nb in enumerate(bank_rows(bk)):
                                tslice[nb] = (t, i)
                        t = tp.tile([128, 3 * 512], BF16, name="tb7", tag="t3")
                        in1 = negs_in1(nrr[:, 24:27, :], 3)
                        dve_max(t, 3, in1, False)
                        for i, nb in enumerate((24, 25, 26)):
                            tslice[nb] = (t, i)

                    def red_matmul(nb, rhs_t, rhs_off, mc, mstart, mstop):
                        bk, jc = bankmap(nb)
                        nc.tensor.matmul(
                            obanks[bk][32 * jc : 32 * jc + 32, :],
                            w2_sb[:, ts(mc, 32)],
                            rhs_t[:, ts(rhs_off, 512)],
                            start=mstart,
                            stop=mstop,
                            tile_position=(0, 32 * jc),
                            skip_group_check=True,
                        )

                    # relu-form matmuls for this red-group's SNBS chunk
                    sc = SNBS_SCHED[c]
                    for nb in SNBS:
                        red_matmul(nb, ta_tiles[(nb, sc)], 0, sc,
                                   c == 0, last)

                    def finish_bank(bk):
                        # split the 8 final psum evacs across both
                        # engines -- both are saturated until the last
                        # max op, so the tail must parallelize
                        if bk in (6, 0, 4):
                            nc.scalar.copy(
                                out=out_sc[:, ts(bk, 512)],
                                in_=obanks[bk][:, :],
                            )
                        else:
                            nc.vector.tensor_copy(
                                out=out_sc[:, ts(bk, 512)],
                                in_=obanks[bk][:, :],
                            )
                        rows = bank_rows(bk)
                        lo, step = rows[0], rows[1] - rows[0]
                        dst = out[:, :].rearrange(
                            "o (nb q) -> (o nb) q", nb=NBLK
                        )[lo : lo + 3 * step + 1 : step, :]
                        src = out_sc[0:128:32, ts(bk, 512)]
                        nc.sync.dma_start(out=dst, in_=src)

                    if not last:
                        for nb in dve_order:
                            t, nbs = tslice[nb]
                            red_matmul(nb, t, nbs, c, c == 0, False)
                    else:
                        # bank 6 (pure relu-form) finished in red-c6;
                        # evac it off the tail, then bank-major pipeline
                        finish_bank(6)
                        for bk in range(6):
                            for nb in bank_rows(bk):
                                if nb in SNBS:
                                    continue
                                t, nbs = tslice[nb]
                                red_matmul(nb, t, nbs, c, False, True)
                            finish_bank(bk)
                        for nb in (24, 25, 26):
                            t, nbs = tslice[nb]
                            red_matmul(nb, t, nbs, c, False, True)
                        finish_bank(7)
                mpsB_cm.__exit__(None, None, None)

    if do_compile:
        nc.compile()
    return nc


_NC_CACHE = None


def _get_nc():
    global _NC_CACHE
    if _NC_CACHE is None:
        _NC_CACHE = _build_nc()
    return _NC_CACHE


def prepare_in_maps(X, Y, W1, b1, W2):
    X = np.asarray(X, dtype=np.float32)
    Y = np.asarray(Y, dtype=np.float32)
    W1 = np.asarray(W1, dtype=np.float32)
    b1 = np.asarray(b1, dtype=np.float32)
    W2 = np.asarray(W2, dtype=np.float32)

    bf = ml_dtypes.bfloat16

    def w1_img(Wh):  # (HD, D) -> flat sbuf image (128, KT*HD)
        return np.ascontiguousarray(
            Wh.reshape(NCH, 128, KT, 128).transpose(3, 0, 2, 1).reshape(128, -1)
        ).astype(bf)

    def xy_img(Xb):  # (N, D) -> flat sbuf image (128, KT*N)
        return np.ascontiguousarray(
            Xb.T.reshape(KT, 128, -1).transpose(1, 0, 2).reshape(128, -1)
        ).astype(bf)

    w1xi = w1_img(W1[:, :D])
    w1yi = w1_img(W1[:, D:])
    CW0 = KT * 128
    b1m = b1.reshape(NCH, 128).T                      # (128, NCH)
    b1cm = np.ascontiguousarray(np.hstack([b1m, -b1m]))  # [+b1 | -b1] f32
    w2cm = np.ascontiguousarray(
        np.repeat(W2.reshape(NCH, 128).T[:, :, None], 32, axis=2).reshape(128, -1)
    ).astype(bf)

    in_maps = []
    for b in range(B):
        in_maps.append(
            {
                "crx": np.ascontiguousarray(
                    np.hstack([xy_img(X[b]), w1xi[:, :CW0]])
                ),
                "cry": np.ascontiguousarray(
                    np.hstack([xy_img(Y[b]), w1yi[:, :CW0]])
                ),
                "w1xt": np.ascontiguousarray(w1xi[:, CW0:]),
                "w1yt": np.ascontiguousarray(w1yi[:, CW0:]),
                "b1c": b1cm,
                "w2c": w2cm,
            }
        )
    return in_maps


def postprocess(raw_outs, X, W1, b1, W2, b2):
    """raw[nb*512 + col]: for DVE rows (nb<27) col = m*4+j; for ScalarE
    rows (nb>=27) col = j*128+m.  DVE rows need the max-form rank-1
    correction gamma (all 8 chunks); ScalarE rows need none."""
    X = np.asarray(X, dtype=np.float32)
    W1 = np.asarray(W1, dtype=np.float32)
    b1 = np.asarray(b1, dtype=np.float32)
    W2 = np.asarray(W2, dtype=np.float32)
    b2 = np.asarray(b2, dtype=np.float32)

    V = (W2[0] @ W1[:, :D]).astype(np.float32)        # (D,)
    g = float(W2[0] @ b1)

    out = np.empty((B, NX, NY), dtype=np.float32)
    for b in range(B):
        r = raw_outs[b].astype(np.float32).reshape(NBLK, 512)
        o = np.empty((NX, NY), dtype=np.float32)
        for nb in range(NBLK):
            if nb < NDVE:
                o[nb * 4 : nb * 4 + 4, :] = (
                    r[nb].reshape(128, 4).T
                )
            else:
                o[nb * 4 : nb * 4 + 4, :] = r[nb].reshape(4, 128)
        gamma = X[b] @ V + g                          # (NX,)
        gamma[NDVE * 4 :] = 0.0
        out[b] = o + gamma[:, None] + b2[0]
    return out


def kernel(X, Y, W1, b1, W2, b2):
    in_maps = prepare_in_maps(X, Y, W1, b1, W2)
    nc = _get_nc()
    res = run_bass_kernel_spmd(nc, in_maps, core_ids=list(range(NCORES)))
    raw = [res.results[b]["out"].reshape(-1) for b in range(B)]
    return postprocess(raw, X, W1, b1, W2, b2)


if __name__ == "__main__":
    rng = np.random.default_rng(0)
    ins = {
        "X": rng.standard_normal((B, NX, D), dtype=np.float32),
        "Y": rng.standard_normal((B, NY, D), dtype=np.float32),
        "W1": rng.standard_normal((HD, 2 * D), dtype=np.float32) * (2 * D) ** -0.5,
        "b1": rng.standard_normal((HD,), dtype=np.float32) * (2 * D) ** -0.5,
        "W2": rng.standard_normal((1, HD), dtype=np.float32) * HD**-0.5,
        "b2": rng.standard_normal((1,), dtype=np.float32) * HD**-0.5,
    }
    o = kernel(**ins)
    print("kernel out:", o.shape, o.dtype, float(np.abs(o).max()))


# revision 13
# speedup vs baseline: 1.0330x; 1.0330x over previous
"""AffinityFC Trainium2 kernel (Bass/Tile, 8 NeuronCores, data-parallel over B).

Math per batch b (one NeuronCore per batch):
    px = X[b] @ W1x.T          (Nx=128, hd=1024)
    py = Y[b] @ W1y.T          (Ny=128, hd=1024)
    out[n, m] = W2 . relu(px[n, :] + py[m, :] + b1) + b2

Key reformulation: with s = px + b1,
    relu(py + s) = max(py, -s) + s
so "max-form" rows compute u = max(py, -s) (one DVE tensor_tensor max
per element at 2x bf16 rate) and reduce Sum_h W2[h]*u with TensorE;
the missing Sum_h W2[h]*s[n,h] term is a rank-1 correction added on
the host.  "relu-form" rows are computed on ScalarE as
relu(py + s[n]) with a per-partition bias, needing no correction.

v21 schedule:
  - Row split is PURE per row-block: nb 0..26 are max-form on DVE in
    every chunk (one w27 TENSOR_TENSOR max per chunk, (m,j)-interleaved
    rhs layout); nb 27..31 are relu-form on ScalarE in every chunk with
    a DENSE (j,m)-block layout (dense src py_plain, dense dst), fully
    decoupled from the DVE chunk loop: the 160 activations stream right
    behind layer-1, their bank-6/7 matmuls run early.
  - PSUM: obanks 0..5 allocated before the layer-1 pool (so their
    matmuls can start immediately); obanks 6,7 reuse layer-1's 2 banks
    after it closes.  Banks 0..5 hold DVE rows {bk, bk+6, bk+12,
    bk+18}; bank 7 holds rows 24,25,26 (DVE) + 27 (ScalarE, jc3);
    bank 6 holds rows 28..31 (ScalarE).
  - Reduction matmuls are issued jc-interleaved so 4 col-groups of the
    PE array run concurrently.
  - DMA: sync queue carries cry=[yt|w1y_c0] then bulk w1y in 2 groups
    then all 8 output DMAs; scalar queue carries crx=[xt|w1x_c0], the
    small consts, then bulk w1x in 2 groups.  Output is bf16 raw.
  - Final psum evacs are split: ScalarE takes banks 0..2 + 6, DVE
    (idle after its last max op) takes banks 3,4,5,7.
  Known hazard: DVE/PE clocks vary between runs (0.96 vs 0.80 GHz);
  compare designs only at equal clock.
"""

import numpy as np
import ml_dtypes

import concourse.mybir as mybir
import concourse.tile as tile
from concourse import bacc
from concourse.bass import ts
from concourse.bass_utils import run_bass_kernel_spmd

B, NX, NY, D, HD = 8, 128, 128, 512, 1024
NCORES = 8
NCH = HD // 128      # 8 h-chunks
KT = D // 128        # 4 k-tiles for the layer-1 contraction
NBLK = NX // 4       # 32 n-blocks of 4 rows each
F32 = mybir.dt.float32
BF16 = mybir.dt.bfloat16

SNBS = (27, 28, 29, 30, 31)   # relu-form rows (ScalarE), all chunks
NDVE = 27                     # nb 0..26 on DVE


def bankmap(nb):  # nb -> (bank, jc)
    if nb < 24:
        return nb % 6, nb // 6
    if nb < 27:
        return 7, nb - 24
    if nb == 27:
        return 7, 3
    return 6, nb - 28


def bank_rows(bk):
    if bk < 6:
        return [bk, bk + 6, bk + 12, bk + 18]
    if bk == 7:
        return [24, 25, 26, 27]
    return [28, 29, 30, 31]


def _build_nc(do_compile=True):
    nc = bacc.Bacc(
        "TRN2", target_bir_lowering=False, debug=False, num_devices=NCORES
    )

    # flat SBUF images: dram[p, col] == sbuf[p, col]
    # crx/cry bundle the chunk-0-critical data into ONE DMA each:
    #   crx = [xt image (KT*NX) | w1x chunk-0 slab (KT*128)]
    CW0 = KT * 128
    crx = nc.dram_tensor("crx", [128, KT * NX + CW0], BF16, kind="ExternalInput")
    cry = nc.dram_tensor("cry", [128, KT * NY + CW0], BF16, kind="ExternalInput")
    w1xt = nc.dram_tensor("w1xt", [128, (NCH - 1) * CW0], BF16, kind="ExternalInput")
    w1yt = nc.dram_tensor("w1yt", [128, (NCH - 1) * CW0], BF16, kind="ExternalInput")
    b1c = nc.dram_tensor("b1c", [128, 2 * NCH], F32, kind="ExternalInput")
    w2c = nc.dram_tensor("w2c", [128, NCH * 32], BF16, kind="ExternalInput")
    out = nc.dram_tensor("out", [1, NBLK * 512], BF16, kind="ExternalOutput")

    with tile.TileContext(nc) as tc:
        with (
            tc.tile_pool(name="const", bufs=1) as cp,
            tc.tile_pool(name="tprod", bufs=4) as tp,
            tc.tile_pool(name="taprod", bufs=12) as tap,
        ):
            crx_sb = cp.tile([128, KT * NX + CW0], BF16)
            cry_sb = cp.tile([128, KT * NY + CW0], BF16)
            xt_sb = crx_sb[:, : KT * NX]
            yt_sb = cry_sb[:, : KT * NY]
            W1GRP = ((1, 4), (4, NCH))
            w1x_g = [
                cp.tile([128, (hi - lo) * KT * 128], BF16, name=f"w1x{lo}")
                for lo, hi in W1GRP
            ]
            w1y_g = [
                cp.tile([128, (hi - lo) * KT * 128], BF16, name=f"w1y{lo}")
                for lo, hi in W1GRP
            ]

            def w1slab(g, c, k):  # lhsT slab for (chunk c, k-tile)
                if c == 0:
                    cr = crx_sb if g is w1x_g else cry_sb
                    off = KT * NX + k * 128
                    return cr[:, off : off + 128]
                for (lo, hi), tile_ in zip(W1GRP, g):
                    if lo <= c < hi:
                        off = ((c - lo) * KT + k) * 128
                        return tile_[:, off : off + 128]
                raise AssertionError

            dummy_sb = cp.tile([128, 64], BF16)     # PE warmup garbage
            b1_sb = cp.tile([128, 2 * NCH], F32)    # [+b1 | -b1] chunk columns
            w2_sb = cp.tile([128, NCH * 32], BF16)
            negs_sb = cp.tile([128, HD], BF16)      # -(px+b1), bf16
            s20_sb = cp.tile([128, NCH * 20], F32)  # s = px+b1 for rows 108..127
            pyr_sb = cp.tile([128, NCH * 512], BF16)  # py rep4 (m,j) per chunk
            pyp_sb = cp.tile([128, NCH * 128], BF16)  # py plain per chunk
            out_sc = cp.tile([128, 8 * 512], BF16)

            # ---- input DMAs.
            # sync queue: cry first (y side gates the first max), then
            # bulk w1y in two groups.  scalar queue: crx, consts, bulk
            # w1x in two groups.  Outputs go on sync at the end.
            nc.sync.dma_start(out=cry_sb[:, :], in_=cry[:, :])
            nc.scalar.dma_start(out=crx_sb[:, :], in_=crx[:, :])
            nc.scalar.dma_start(out=b1_sb[:, :], in_=b1c[:, :])
            nc.scalar.dma_start(out=w2_sb[:, :], in_=w2c[:, :])
            # bulk W1 split across both queues, interleaved by deadline:
            # sync: y(1-3), x(4-7); scalar: x(1-3), y(4-7)
            nc.sync.dma_start(out=w1y_g[0][:, :], in_=w1yt[:, : 3 * CW0])
            nc.scalar.dma_start(out=w1x_g[0][:, :], in_=w1xt[:, : 3 * CW0])
            nc.sync.dma_start(out=w1x_g[1][:, :], in_=w1xt[:, 3 * CW0 :])
            nc.scalar.dma_start(out=w1y_g[1][:, :], in_=w1yt[:, 3 * CW0 :])

            with tc.tile_pool(name="mpsA", bufs=1, space="PSUM") as mpsA:
                obanks = [None] * 8
                for i in range(6):
                    obanks[i] = mpsA.tile([128, 512], F32, name=f"ob{i}", tag=f"ob{i}")

                # PE warmup: ~30 no-input matmuls flip the HAM clock gate
                # to 8/8 before the first real layer-1 matmuls arrive.
                nc.gpsimd.memset(dummy_sb[:, :], 0.0)
                for _ in range(30):
                    nc.tensor.matmul(
                        obanks[0][0:64, 0:64],
                        dummy_sb[:, :],
                        dummy_sb[:, :],
                        start=True,
                        stop=True,
                        skip_group_check=True,
                    )

                # ---- layer 1 per h-chunk + the full ScalarE act stream.
                # ScalarE FIFO order: [c evacs + acts] for c=0..2, then
                # evacs-only for c=3..7, then c7 acts (so the bank-6/7
                # relu-form accumulation can finish off the tail), then
                # the deferred c3..c6 acts.
                ta_tiles = {}  # (nb, c) -> tile
                deferred_act_cs = []

                def emit_acts(c):
                    for nb in SNBS:
                        ta = tap.tile(
                            [128, 512], BF16, name=f"ta{c}_{nb}", tag="ta"
                        )
                        ta_tiles[(nb, c)] = ta
                        for j in range(4):
                            n = nb * 4 + j
                            nc.scalar.activation(
                                out=ta[:, ts(j, 128)],
                                in_=pyp_sb[:, ts(c, 128)],
                                func=mybir.ActivationFunctionType.Relu,
                                bias=s20_sb[:, c * 20 + n - 108 : c * 20 + n - 107],
                                scale=1.0,
                            )

                with tc.tile_pool(name="l1ps", bufs=2, space="PSUM") as l1ps:
                    for c in range(NCH):
                        pyp = l1ps.tile([128, NY], F32, tag="l1")
                        for k in range(KT):
                            nc.tensor.matmul(
                                pyp[:, :],
                                w1slab(w1y_g, c, k),
                                yt_sb[:, ts(k, NY)],
                                start=(k == 0),
                                stop=(k == KT - 1),
                            )
                        with tc.high_priority():
                            nc.scalar.activation(
                                out=pyr_sb[:, ts(c, 512)].rearrange(
                                    "p (m j) -> p m j", j=4
                                ),
                                in_=pyp[:, :].unsqueeze(2).broadcast_to(
                                    (128, 128, 4)
                                ),
                                func=mybir.ActivationFunctionType.Copy,
                            )
                            nc.scalar.copy(
                                out=pyp_sb[:, ts(c, 128)], in_=pyp[:, :]
                            )
                        pxp = l1ps.tile([128, NX], F32, tag="l1")
                        for k in range(KT):
                            nc.tensor.matmul(
                                pxp[:, :],
                                w1slab(w1x_g, c, k),
                                xt_sb[:, ts(k, NX)],
                                start=(k == 0),
                                stop=(k == KT - 1),
                            )
                        with tc.high_priority():
                            nc.scalar.activation(
                                out=negs_sb[:, ts(c, 128)],
                                in_=pxp[:, :],
                                func=mybir.ActivationFunctionType.Identity,
                                bias=b1_sb[:, NCH + c : NCH + c + 1],
                                scale=-1.0,
                            )
                            nc.scalar.activation(
                                out=s20_sb[:, ts(c, 20)],
                                in_=pxp[:, 108:128],
                                func=mybir.ActivationFunctionType.Identity,
                                bias=b1_sb[:, c : c + 1],
                                scale=1.0,
                            )
                        if c == 0:
                            # bulk w1x DMA issues ride behind the chunk-0
                            # evacs on the Scalar engine
                            nc.scalar.dma_start(
                                out=w1x_g[0][:, :], in_=w1xt[:, : 3 * CW0]
                            )
                            nc.scalar.dma_start(
                                out=w1x_g[1][:, :], in_=w1xt[:, 3 * CW0 :]
                            )
                        if c <= 2:
                            emit_acts(c)
                        elif c < NCH - 1:
                            deferred_act_cs.append(c)
                        else:
                            emit_acts(c)            # c7 acts hoisted
                            for dc in deferred_act_cs:
                                emit_acts(dc)

                # banks 6,7 reuse layer-1's psum space (deps via allocator)
                mpsB_cm = tc.tile_pool(name="mpsB", bufs=1, space="PSUM")
                mpsB = mpsB_cm.__enter__()
                obanks[6] = mpsB.tile([128, 512], F32, name="ob6", tag="ob6")
                obanks[7] = mpsB.tile([128, 512], F32, name="ob7", tag="ob7")

                # jc-interleaved issue order for the 27 DVE rows (4-way
                # col-group concurrency on the PE array)
                dve_order = []
                for r in range(6):
                    for j in range(4):
                        dve_order.append(6 * j + ((r + j) % 6))
                dve_order = [nb for nb in dve_order if nb < 24]
                dve_order += [24, 25, 26]

                # SNBS (relu-form) matmuls are issued one per red-group;
                # chunk 7's acts are hoisted so its matmul can run early,
                # and the stop flag goes on the last-issued chunk (c6 —
                # accumulation order is irrelevant, adds commute).
                SNBS_SCHED = [0, 1, 2, 7, 3, 4, 5, 6]

                # ---- main loop, c-outer
                for c in range(NCH):
                    last = c == NCH - 1
                    pyr_c = pyr_sb[:, ts(c, 512)]
                    pyr3 = pyr_c.rearrange("p (m j) -> p m j", j=4)
                    tslice = {}  # nb -> (tile, column offset index)

                    def dve_max(t, w, in1, prio):
                        in0 = pyr3.unsqueeze(1).broadcast_to((128, w, 128, 4))
                        out_ap = t[:, :].rearrange(
                            "p (nbs m j) -> p nbs m j", nbs=w, m=128
                        )
                        if prio:
                            with tc.high_priority():
                                nc.vector.tensor_tensor(
                                    out=out_ap, in0=in0, in1=in1,
                                    op=mybir.AluOpType.max,
                                )
                        else:
                            nc.vector.tensor_tensor(
                                out=out_ap, in0=in0, in1=in1,
                                op=mybir.AluOpType.max,
                            )

                    def negs_in1(cols, w):
                        # cols: list-slice of negs columns [p, w, 4] -> bcast m
                        return cols.unsqueeze(2).broadcast_to((128, w, 128, 4))

                    if not last:
                        t = tp.tile([128, NDVE * 512], BF16, name=f"t{c}",
                                    tag="t")
                        in1 = negs_in1(
                            negs_sb[:, c * 128 : c * 128 + 4 * NDVE]
                            .rearrange("p (nbs j) -> p nbs j", j=4), NDVE
                        )
                        dve_max(t, NDVE, in1, c == 0)
                        for nbs in range(NDVE):
                            tslice[nbs] = (t, nbs)
                    else:
                        # bank-grouped ops so the final bank-major
                        # matmul+evac+DMA pipeline starts per-bank
                        nrr = negs_sb[:, ts(c, 128)].rearrange(
                            "p (nb j) -> p nb j", j=4
                        )
                        for bk in range(6):
                            t = tp.tile(
                                [128, 4 * 512], BF16, name=f"tb{bk}",
                                tag="t4", bufs=6,
                            )
                            in1 = negs_in1(
                                nrr[:, bk : bk + 19 : 6, :], 4
                            )
                            dve_max(t, 4, in1, False)
                            for i, nb in enumerate(bank_rows(bk)):
                                tslice[nb] = (t, i)
                        t = tp.tile([128, 3 * 512], BF16, name="tb7", tag="t3")
                        in1 = negs_in1(nrr[:, 24:27, :], 3)
                        dve_max(t, 3, in1, False)
                        for i, nb in enumerate((24, 25, 26)):
                            tslice[nb] = (t, i)

                    def red_matmul(nb, rhs_t, rhs_off, mc, mstart, mstop):
                        bk, jc = bankmap(nb)
                        nc.tensor.matmul(
                            obanks[bk][32 * jc : 32 * jc + 32, :],
                            w2_sb[:, ts(mc, 32)],
                            rhs_t[:, ts(rhs_off, 512)],
                            start=mstart,
                            stop=mstop,
                            tile_position=(0, 32 * jc),
                            skip_group_check=True,
                        )

                    sc = SNBS_SCHED[c]

                    def finish_bank(bk):
                        # split the final psum evacs + out-DMA issues
                        # across both engines/queues -- both are
                        # saturated until the last max op, so the tail
                        # must parallelize
                        if bk in (6, 0, 4):
                            nc.scalar.copy(
                                out=out_sc[:, ts(bk, 512)],
                                in_=obanks[bk][:, :],
                            )
                            dq = nc.scalar
                        else:
                            nc.vector.tensor_copy(
                                out=out_sc[:, ts(bk, 512)],
                                in_=obanks[bk][:, :],
                            )
                            dq = nc.sync
                        rows = bank_rows(bk)
                        lo, step = rows[0], rows[1] - rows[0]
                        dst = out[:, :].rearrange(
                            "o (nb q) -> (o nb) q", nb=NBLK
                        )[lo : lo + 3 * step + 1 : step, :]
                        src = out_sc[0:128:32, ts(bk, 512)]
                        dq.dma_start(out=dst, in_=src)

                    if not last:
                        # relu-form matmuls for this red-group's chunk
                        for nb in SNBS:
                            red_matmul(nb, ta_tiles[(nb, sc)], 0, sc,
                                       c == 0, False)
                        for nb in dve_order:
                            t, nbs = tslice[nb]
                            red_matmul(nb, t, nbs, c, c == 0, False)
                    else:
                        # bank-major pipeline for the DVE banks first;
                        # the SNBS stop matmuls (gated by the act-stream
                        # end) must not block it in the PE FIFO
                        for bk in range(6):
                            for nb in bank_rows(bk):
                                t, nbs = tslice[nb]
                                red_matmul(nb, t, nbs, c, False, True)
                            finish_bank(bk)
                        for nb in SNBS:
                            red_matmul(nb, ta_tiles[(nb, sc)], 0, sc,
                                       False, True)
                        finish_bank(6)
                        for nb in (24, 25, 26):
                            t, nbs = tslice[nb]
                            red_matmul(nb, t, nbs, c, False, True)
                        finish_bank(7)
                mpsB_cm.__exit__(None, None, None)

    if do_compile:
        nc.compile()
    return nc


_NC_CACHE = None


def _get_nc():
    global _NC_CACHE
    if _NC_CACHE is None:
        _NC_CACHE = _build_nc()
    return _NC_CACHE


def prepare_in_maps(X, Y, W1, b1, W2):
    X = np.asarray(X, dtype=np.float32)
    Y = np.asarray(Y, dtype=np.float32)
    W1 = np.asarray(W1, dtype=np.float32)
    b1 = np.asarray(b1, dtype=np.float32)
    W2 = np.asarray(W2, dtype=np.float32)

    bf = ml_dtypes.bfloat16

    def w1_img(Wh):  # (HD, D) -> flat sbuf image (128, KT*HD)
        return np.ascontiguousarray(
            Wh.reshape(NCH, 128, KT, 128).transpose(3, 0, 2, 1).reshape(128, -1)
        ).astype(bf)

    def xy_img(Xb):  # (N, D) -> flat sbuf image (128, KT*N)
        return np.ascontiguousarray(
            Xb.T.reshape(KT, 128, -1).transpose(1, 0, 2).reshape(128, -1)
        ).astype(bf)

    w1xi = w1_img(W1[:, :D])
    w1yi = w1_img(W1[:, D:])
    CW0 = KT * 128
    b1m = b1.reshape(NCH, 128).T                      # (128, NCH)
    b1cm = np.ascontiguousarray(np.hstack([b1m, -b1m]))  # [+b1 | -b1] f32
    w2cm = np.ascontiguousarray(
        np.repeat(W2.reshape(NCH, 128).T[:, :, None], 32, axis=2).reshape(128, -1)
    ).astype(bf)

    in_maps = []
    for b in range(B):
        in_maps.append(
            {
                "crx": np.ascontiguousarray(
                    np.hstack([xy_img(X[b]), w1xi[:, :CW0]])
                ),
                "cry": np.ascontiguousarray(
                    np.hstack([xy_img(Y[b]), w1yi[:, :CW0]])
                ),
                "w1xt": np.ascontiguousarray(w1xi[:, CW0:]),
                "w1yt": np.ascontiguousarray(w1yi[:, CW0:]),
                "b1c": b1cm,
                "w2c": w2cm,
            }
        )
    return in_maps


def postprocess(raw_outs, X, W1, b1, W2, b2):
    """raw[nb*512 + col]: for DVE rows (nb<27) col = m*4+j; for ScalarE
    rows (nb>=27) col = j*128+m.  DVE rows need the max-form rank-1
    correction gamma (all 8 chunks); ScalarE rows need none."""
    X = np.asarray(X, dtype=np.float32)
    W1 = np.asarray(W1, dtype=np.float32)
    b1 = np.asarray(b1, dtype=np.float32)
    W2 = np.asarray(W2, dtype=np.float32)
    b2 = np.asarray(b2, dtype=np.float32)

    V = (W2[0] @ W1[:, :D]).astype(np.float32)        # (D,)
    g = float(W2[0] @ b1)

    out = np.empty((B, NX, NY), dtype=np.float32)
    for b in range(B):
        r = raw_outs[b].astype(np.float32).reshape(NBLK, 512)
        o = np.empty((NX, NY), dtype=np.float32)
        for nb in range(NBLK):
            if nb < NDVE:
                o[nb * 4 : nb * 4 + 4, :] = (
                    r[nb].reshape(128, 4).T
                )
            else:
                o[nb * 4 : nb * 4 + 4, :] = r[nb].reshape(4, 128)
        gamma = X[b] @ V + g                          # (NX,)
        gamma[NDVE * 4 :] = 0.0
        out[b] = o + gamma[:, None] + b2[0]
    return out


def kernel(X, Y, W1, b1, W2, b2):
    in_maps = prepare_in_maps(X, Y, W1, b1, W2)
    nc = _get_nc()
    res = run_bass_kernel_spmd(nc, in_maps, core_ids=list(range(NCORES)))
    raw = [res.results[b]["out"].reshape(-1) for b in range(B)]
    return postprocess(raw, X, W1, b1, W2, b2)


if __name__ == "__main__":
    rng = np.random.default_rng(0)
    ins = {
        "X": rng.standard_normal((B, NX, D), dtype=np.float32),
        "Y": rng.standard_normal((B, NY, D), dtype=np.float32),
        "W1": rng.standard_normal((HD, 2 * D), dtype=np.float32) * (2 * D) ** -0.5,
        "b1": rng.standard_normal((HD,), dtype=np.float32) * (2 * D) ** -0.5,
        "W2": rng.standard_normal((1, HD), dtype=np.float32) * HD**-0.5,
        "b2": rng.standard_normal((1,), dtype=np.float32) * HD**-0.5,
    }
    o = kernel(**ins)
    print("kernel out:", o.shape, o.dtype, float(np.abs(o).max()))


# revision 17
# speedup vs baseline: 1.0616x; 1.0277x over previous
"""AffinityFC Trainium2 kernel (Bass/Tile, 8 NeuronCores, data-parallel over B).

Math per batch b (one NeuronCore per batch):
    px = X[b] @ W1x.T          (Nx=128, hd=1024)
    py = Y[b] @ W1y.T          (Ny=128, hd=1024)
    out[n, m] = W2 . relu(px[n, :] + py[m, :] + b1) + b2

Key reformulation: with s = px + b1,
    relu(py + s) = max(py, -s) + s
so "max-form" rows compute u = max(py, -s) (one DVE tensor_tensor max
per element at 2x bf16 rate) and reduce Sum_h W2[h]*u with TensorE;
the missing Sum_h W2[h]*s[n,h] term is a rank-1 correction added on
the host.  "relu-form" rows are computed on ScalarE as
relu(py + s[n]) with a per-partition bias, needing no correction.

v21 schedule:
  - Row split is PURE per row-block: nb 0..26 are max-form on DVE in
    every chunk (one w27 TENSOR_TENSOR max per chunk, (m,j)-interleaved
    rhs layout); nb 27..31 are relu-form on ScalarE in every chunk with
    a DENSE (j,m)-block layout (dense src py_plain, dense dst), fully
    decoupled from the DVE chunk loop: the 160 activations stream right
    behind layer-1, their bank-6/7 matmuls run early.
  - PSUM: obanks 0..5 allocated before the layer-1 pool (so their
    matmuls can start immediately); obanks 6,7 reuse layer-1's 2 banks
    after it closes.  Banks 0..5 hold DVE rows {bk, bk+6, bk+12,
    bk+18}; bank 7 holds rows 24,25,26 (DVE) + 27 (ScalarE, jc3);
    bank 6 holds rows 28..31 (ScalarE).
  - Reduction matmuls are issued jc-interleaved so 4 col-groups of the
    PE array run concurrently.
  - DMA: sync queue carries cry=[yt|w1y_c0] then bulk w1y in 2 groups
    then all 8 output DMAs; scalar queue carries crx=[xt|w1x_c0], the
    small consts, then bulk w1x in 2 groups.  Output is bf16 raw.
  - Final psum evacs are split: ScalarE takes banks 0..2 + 6, DVE
    (idle after its last max op) takes banks 3,4,5,7.
  Known hazard: DVE/PE clocks vary between runs (0.96 vs 0.80 GHz);
  compare designs only at equal clock.
"""

import numpy as np
import ml_dtypes

import concourse.mybir as mybir
import concourse.tile as tile
from concourse import bacc
from concourse.bass import ts
from concourse.bass_utils import run_bass_kernel_spmd

B, NX, NY, D, HD = 8, 128, 128, 512, 1024
NCORES = 8
NCH = HD // 128      # 8 h-chunks
KT = D // 128        # 4 k-tiles for the layer-1 contraction
NBLK = NX // 4       # 32 n-blocks of 4 rows each
F32 = mybir.dt.float32
BF16 = mybir.dt.bfloat16

SNBS = (27, 28, 29, 30, 31)   # relu-form rows (ScalarE), all chunks
NDVE = 27                     # nb 0..26 on DVE


def bankmap(nb):  # nb -> (bank, jc)
    if nb < 24:
        return nb % 6, nb // 6
    if nb < 27:
        return 7, nb - 24
    if nb == 27:
        return 7, 3
    return 6, nb - 28


def bank_rows(bk):
    if bk < 6:
        return [bk, bk + 6, bk + 12, bk + 18]
    if bk == 7:
        return [24, 25, 26, 27]
    return [28, 29, 30, 31]


def _build_nc(do_compile=True):
    nc = bacc.Bacc(
        "TRN2", target_bir_lowering=False, debug=False, num_devices=NCORES
    )

    # flat SBUF images: dram[p, col] == sbuf[p, col]
    # crx/cry bundle the chunk-0-critical data into ONE DMA each:
    #   crx = [xt image (KT*NX) | w1x chunk-0 slab (KT*128)]
    CW0 = KT * 128
    crx = nc.dram_tensor("crx", [128, KT * NX + CW0], BF16, kind="ExternalInput")
    cry = nc.dram_tensor("cry", [128, KT * NY + CW0], BF16, kind="ExternalInput")
    w1xt = nc.dram_tensor("w1xt", [128, (NCH - 1) * CW0], BF16, kind="ExternalInput")
    w1yt = nc.dram_tensor("w1yt", [128, (NCH - 1) * CW0], BF16, kind="ExternalInput")
    b1c = nc.dram_tensor("b1c", [128, 2 * NCH], F32, kind="ExternalInput")
    w2c = nc.dram_tensor("w2c", [128, NCH * 32], BF16, kind="ExternalInput")
    out = nc.dram_tensor("out", [1, NBLK * 512], BF16, kind="ExternalOutput")

    with tile.TileContext(nc) as tc:
        with (
            tc.tile_pool(name="const", bufs=1) as cp,
            tc.tile_pool(name="tprod", bufs=4) as tp,
            tc.tile_pool(name="taprod", bufs=12) as tap,
        ):
            crx_sb = cp.tile([128, KT * NX + CW0], BF16)
            cry_sb = cp.tile([128, KT * NY + CW0], BF16)
            xt_sb = crx_sb[:, : KT * NX]
            yt_sb = cry_sb[:, : KT * NY]
            W1GRP = ((1, 4), (4, NCH))
            w1x_g = [
                cp.tile([128, (hi - lo) * KT * 128], BF16, name=f"w1x{lo}")
                for lo, hi in W1GRP
            ]
            w1y_g = [
                cp.tile([128, (hi - lo) * KT * 128], BF16, name=f"w1y{lo}")
                for lo, hi in W1GRP
            ]

            def w1slab(g, c, k):  # lhsT slab for (chunk c, k-tile)
                if c == 0:
                    cr = crx_sb if g is w1x_g else cry_sb
                    off = KT * NX + k * 128
                    return cr[:, off : off + 128]
                for (lo, hi), tile_ in zip(W1GRP, g):
                    if lo <= c < hi:
                        off = ((c - lo) * KT + k) * 128
                        return tile_[:, off : off + 128]
                raise AssertionError

            dummy_sb = cp.tile([128, 64], BF16)     # PE warmup garbage
            b1_sb = cp.tile([128, 2 * NCH], F32)    # [+b1 | -b1] chunk columns
            w2_sb = cp.tile([128, NCH * 32], BF16)
            negs_sb = cp.tile([128, HD], BF16)      # -(px+b1), bf16
            s20_sb = cp.tile([128, NCH * 20], F32)  # s = px+b1 for rows 108..127
            pyr_sb = cp.tile([128, NCH * 512], BF16)  # py rep4 (m,j) per chunk
            pyp_sb = cp.tile([128, NCH * 128], BF16)  # py plain per chunk
            out_sc = cp.tile([128, 8 * 512], BF16)

            # ---- input DMAs.
            # sync queue: cry first (y side gates the first max), then
            # bulk w1y in two groups.  scalar queue: crx, consts, bulk
            # w1x in two groups.  Outputs go on sync at the end.
            nc.sync.dma_start(out=cry_sb[:, :], in_=cry[:, :])
            nc.scalar.dma_start(out=crx_sb[:, :], in_=crx[:, :])
            nc.scalar.dma_start(out=b1_sb[:, :], in_=b1c[:, :])
            nc.scalar.dma_start(out=w2_sb[:, :], in_=w2c[:, :])
            # bulk W1: y side on sync, x side on scalar (few DMAs per
            # queue -- more would recycle completion semaphores and
            # stall the issuing engine on the previous transfer)
            nc.sync.dma_start(out=w1y_g[0][:, :], in_=w1yt[:, : 3 * CW0])
            nc.scalar.dma_start(out=w1x_g[0][:, :], in_=w1xt[:, : 3 * CW0])
            nc.sync.dma_start(out=w1y_g[1][:, :], in_=w1yt[:, 3 * CW0 :])
            nc.scalar.dma_start(out=w1x_g[1][:, :], in_=w1xt[:, 3 * CW0 :])

            with tc.tile_pool(name="mpsA", bufs=1, space="PSUM") as mpsA:
                obanks = [None] * 8
                for i in range(6):
                    obanks[i] = mpsA.tile([128, 512], F32, name=f"ob{i}", tag=f"ob{i}")

                # PE warmup: ~30 no-input matmuls flip the HAM clock gate
                # to 8/8 before the first real layer-1 matmuls arrive.
                nc.gpsimd.memset(dummy_sb[:, :], 0.0)
                for _ in range(30):
                    nc.tensor.matmul(
                        obanks[0][0:64, 0:64],
                        dummy_sb[:, :],
                        dummy_sb[:, :],
                        start=True,
                        stop=True,
                        skip_group_check=True,
                    )

                # ---- layer 1 per h-chunk + the full ScalarE act stream.
                # ScalarE FIFO order: [c evacs + acts] for c=0..2, then
                # evacs-only for c=3..7, then c7 acts (so the bank-6/7
                # relu-form accumulation can finish off the tail), then
                # the deferred c3..c6 acts.
                ta_tiles = {}  # (nb, c) -> tile
                deferred_act_cs = []

                def emit_acts(c):
                    for nb in SNBS:
                        ta = tap.tile(
                            [128, 512], BF16, name=f"ta{c}_{nb}", tag="ta"
                        )
                        ta_tiles[(nb, c)] = ta
                        for j in range(4):
                            n = nb * 4 + j
                            nc.scalar.activation(
                                out=ta[:, ts(j, 128)],
                                in_=pyp_sb[:, ts(c, 128)],
                                func=mybir.ActivationFunctionType.Relu,
                                bias=s20_sb[:, c * 20 + n - 108 : c * 20 + n - 107],
                                scale=1.0,
                            )

                with tc.tile_pool(name="l1ps", bufs=2, space="PSUM") as l1ps:
                    for c in range(NCH):
                        pyp = l1ps.tile([128, NY], F32, tag="l1")
                        for k in range(KT):
                            nc.tensor.matmul(
                                pyp[:, :],
                                w1slab(w1y_g, c, k),
                                yt_sb[:, ts(k, NY)],
                                start=(k == 0),
                                stop=(k == KT - 1),
                            )
                        with tc.high_priority():
                            nc.scalar.activation(
                                out=pyr_sb[:, ts(c, 512)].rearrange(
                                    "p (m j) -> p m j", j=4
                                ),
                                in_=pyp[:, :].unsqueeze(2).broadcast_to(
                                    (128, 128, 4)
                                ),
                                func=mybir.ActivationFunctionType.Copy,
                            )
                            nc.scalar.copy(
                                out=pyp_sb[:, ts(c, 128)], in_=pyp[:, :]
                            )
                        pxp = l1ps.tile([128, NX], F32, tag="l1")
                        for k in range(KT):
                            nc.tensor.matmul(
                                pxp[:, :],
                                w1slab(w1x_g, c, k),
                                xt_sb[:, ts(k, NX)],
                                start=(k == 0),
                                stop=(k == KT - 1),
                            )
                        with tc.high_priority():
                            nc.scalar.activation(
                                out=negs_sb[:, ts(c, 128)],
                                in_=pxp[:, :],
                                func=mybir.ActivationFunctionType.Identity,
                                bias=b1_sb[:, NCH + c : NCH + c + 1],
                                scale=-1.0,
                            )
                            nc.scalar.activation(
                                out=s20_sb[:, ts(c, 20)],
                                in_=pxp[:, 108:128],
                                func=mybir.ActivationFunctionType.Identity,
                                bias=b1_sb[:, c : c + 1],
                                scale=1.0,
                            )
                        if c == 0:
                            # bulk w1x DMA issues ride behind the chunk-0
                            # evacs on the Scalar engine
                            nc.scalar.dma_start(
                                out=w1x_g[0][:, :], in_=w1xt[:, : 3 * CW0]
                            )
                            nc.scalar.dma_start(
                                out=w1x_g[1][:, :], in_=w1xt[:, 3 * CW0 :]
                            )
                        if c <= 2:
                            emit_acts(c)
                        elif c < NCH - 1:
                            deferred_act_cs.append(c)
                        else:
                            emit_acts(c)            # c7 acts hoisted
                            for dc in deferred_act_cs:
                                emit_acts(dc)

                # banks 6,7 reuse layer-1's psum space (deps via allocator)
                mpsB_cm = tc.tile_pool(name="mpsB", bufs=1, space="PSUM")
                mpsB = mpsB_cm.__enter__()
                obanks[6] = mpsB.tile([128, 512], F32, name="ob6", tag="ob6")
                obanks[7] = mpsB.tile([128, 512], F32, name="ob7", tag="ob7")

                # jc-interleaved issue order for the 27 DVE rows (4-way
                # col-group concurrency on the PE array)
                dve_order = []
                for r in range(6):
                    for j in range(4):
                        dve_order.append(6 * j + ((r + j) % 6))
                dve_order = [nb for nb in dve_order if nb < 24]
                dve_order += [24, 25, 26]

                # SNBS (relu-form) matmuls are issued one per red-group;
                # chunk 7's acts are hoisted so its matmul can run early,
                # and the stop flag goes on the last-issued chunk (c6 —
                # accumulation order is irrelevant, adds commute).
                SNBS_SCHED = [0, 1, 2, 7, 3, 4, 5, 6]

                # ---- main loop, c-outer
                for c in range(NCH):
                    last = c == NCH - 1
                    pyr_c = pyr_sb[:, ts(c, 512)]
                    pyr3 = pyr_c.rearrange("p (m j) -> p m j", j=4)
                    tslice = {}  # nb -> (tile, column offset index)

                    def dve_max(t, w, in1, prio):
                        in0 = pyr3.unsqueeze(1).broadcast_to((128, w, 128, 4))
                        out_ap = t[:, :].rearrange(
                            "p (nbs m j) -> p nbs m j", nbs=w, m=128
                        )
                        if prio:
                            with tc.high_priority():
                                nc.vector.tensor_tensor(
                                    out=out_ap, in0=in0, in1=in1,
                                    op=mybir.AluOpType.max,
                                )
                        else:
                            nc.vector.tensor_tensor(
                                out=out_ap, in0=in0, in1=in1,
                                op=mybir.AluOpType.max,
                            )

                    def negs_in1(cols, w):
                        # cols: list-slice of negs columns [p, w, 4] -> bcast m
                        return cols.unsqueeze(2).broadcast_to((128, w, 128, 4))

                    if not last:
                        t = tp.tile([128, NDVE * 512], BF16, name=f"t{c}",
                                    tag="t")
                        in1 = negs_in1(
                            negs_sb[:, c * 128 : c * 128 + 4 * NDVE]
                            .rearrange("p (nbs j) -> p nbs j", j=4), NDVE
                        )
                        dve_max(t, NDVE, in1, c == 0)
                        for nbs in range(NDVE):
                            tslice[nbs] = (t, nbs)
                    else:
                        # bank-grouped ops so the final bank-major
                        # matmul+evac+DMA pipeline starts per-bank
                        nrr = negs_sb[:, ts(c, 128)].rearrange(
                            "p (nb j) -> p nb j", j=4
                        )
                        for bk in range(6):
                            t = tp.tile(
                                [128, 4 * 512], BF16, name=f"tb{bk}",
                                tag="t4", bufs=6,
                            )
                            in1 = negs_in1(
                                nrr[:, bk : bk + 19 : 6, :], 4
                            )
                            dve_max(t, 4, in1, False)
                            for i, nb in enumerate(bank_rows(bk)):
                                tslice[nb] = (t, i)
                        t = tp.tile([128, 3 * 512], BF16, name="tb7", tag="t3")
                        in1 = negs_in1(nrr[:, 24:27, :], 3)
                        dve_max(t, 3, in1, False)
                        for i, nb in enumerate((24, 25, 26)):
                            tslice[nb] = (t, i)

                    def red_matmul(nb, rhs_t, rhs_off, mc, mstart, mstop):
                        bk, jc = bankmap(nb)
                        nc.tensor.matmul(
                            obanks[bk][32 * jc : 32 * jc + 32, :],
                            w2_sb[:, ts(mc, 32)],
                            rhs_t[:, ts(rhs_off, 512)],
                            start=mstart,
                            stop=mstop,
                            tile_position=(0, 32 * jc),
                            skip_group_check=True,
                        )

                    sc = SNBS_SCHED[c]

                    def finish_bank(bk):
                        # split the final psum evacs across both engines
                        # -- both are saturated until the last max op,
                        # so the tail must parallelize.  The out-DMAs
                        # are merged into two (one per queue) below.
                        if bk in (6, 0, 4):
                            nc.scalar.copy(
                                out=out_sc[:, ts(bk, 512)],
                                in_=obanks[bk][:, :],
                            )
                        else:
                            nc.vector.tensor_copy(
                                out=out_sc[:, ts(bk, 512)],
                                in_=obanks[bk][:, :],
                            )

                    if not last:
                        # relu-form matmuls for this red-group's chunk
                        for nb in SNBS:
                            red_matmul(nb, ta_tiles[(nb, sc)], 0, sc,
                                       c == 0, False)
                        for nb in dve_order:
                            t, nbs = tslice[nb]
                            red_matmul(nb, t, nbs, c, c == 0, False)
                    else:
                        # bank-major pipeline for the DVE banks first;
                        # the SNBS stop matmuls (gated by the act-stream
                        # end) must not block it in the PE FIFO
                        for bk in range(6):
                            for nb in bank_rows(bk):
                                t, nbs = tslice[nb]
                                red_matmul(nb, t, nbs, c, False, True)
                            finish_bank(bk)
                        for nb in SNBS:
                            red_matmul(nb, ta_tiles[(nb, sc)], 0, sc,
                                       False, True)
                        finish_bank(6)
                        for nb in (24, 25, 26):
                            t, nbs = tslice[nb]
                            red_matmul(nb, t, nbs, c, False, True)
                        finish_bank(7)
                        # two merged out-DMAs: raw layout is the out_sc
                        # image itself: [half h][jc 0..3][bank bk%4][512]
                        for h, dq in ((0, nc.sync), (1, nc.scalar)):
                            dst = out[:, h * 8192 : (h + 1) * 8192].rearrange(
                                "o (p q) -> (o p) q", p=4
                            )
                            src = out_sc[0:128:32, h * 2048 : (h + 1) * 2048]
                            dq.dma_start(out=dst, in_=src)
                mpsB_cm.__exit__(None, None, None)

    if do_compile:
        nc.compile()
    return nc


_NC_CACHE = None


def _get_nc():
    global _NC_CACHE
    if _NC_CACHE is None:
        _NC_CACHE = _build_nc()
    return _NC_CACHE


def prepare_in_maps(X, Y, W1, b1, W2):
    X = np.asarray(X, dtype=np.float32)
    Y = np.asarray(Y, dtype=np.float32)
    W1 = np.asarray(W1, dtype=np.float32)
    b1 = np.asarray(b1, dtype=np.float32)
    W2 = np.asarray(W2, dtype=np.float32)

    bf = ml_dtypes.bfloat16

    def w1_img(Wh):  # (HD, D) -> flat sbuf image (128, KT*HD)
        return np.ascontiguousarray(
            Wh.reshape(NCH, 128, KT, 128).transpose(3, 0, 2, 1).reshape(128, -1)
        ).astype(bf)

    def xy_img(Xb):  # (N, D) -> flat sbuf image (128, KT*N)
        return np.ascontiguousarray(
            Xb.T.reshape(KT, 128, -1).transpose(1, 0, 2).reshape(128, -1)
        ).astype(bf)

    w1xi = w1_img(W1[:, :D])
    w1yi = w1_img(W1[:, D:])
    CW0 = KT * 128
    b1m = b1.reshape(NCH, 128).T                      # (128, NCH)
    b1cm = np.ascontiguousarray(np.hstack([b1m, -b1m]))  # [+b1 | -b1] f32
    w2cm = np.ascontiguousarray(
        np.repeat(W2.reshape(NCH, 128).T[:, :, None], 32, axis=2).reshape(128, -1)
    ).astype(bf)

    in_maps = []
    for b in range(B):
        in_maps.append(
            {
                "crx": np.ascontiguousarray(
                    np.hstack([xy_img(X[b]), w1xi[:, :CW0]])
                ),
                "cry": np.ascontiguousarray(
                    np.hstack([xy_img(Y[b]), w1yi[:, :CW0]])
                ),
                "w1xt": np.ascontiguousarray(w1xi[:, CW0:]),
                "w1yt": np.ascontiguousarray(w1yi[:, CW0:]),
                "b1c": b1cm,
                "w2c": w2cm,
            }
        )
    return in_maps


def postprocess(raw_outs, X, W1, b1, W2, b2):
    """raw[nb*512 + col]: for DVE rows (nb<27) col = m*4+j; for ScalarE
    rows (nb>=27) col = j*128+m.  DVE rows need the max-form rank-1
    correction gamma (all 8 chunks); ScalarE rows need none."""
    X = np.asarray(X, dtype=np.float32)
    W1 = np.asarray(W1, dtype=np.float32)
    b1 = np.asarray(b1, dtype=np.float32)
    W2 = np.asarray(W2, dtype=np.float32)
    b2 = np.asarray(b2, dtype=np.float32)

    V = (W2[0] @ W1[:, :D]).astype(np.float32)        # (D,)
    g = float(W2[0] @ b1)

    out = np.empty((B, NX, NY), dtype=np.float32)
    for b in range(B):
        # raw layout: [half h][jc 0..3][bank bk%4][512] (out_sc image)
        r = raw_outs[b].astype(np.float32).reshape(2, 4, 4, 512)
        o = np.empty((NX, NY), dtype=np.float32)
        for bk in range(8):
            for jc in range(4):
                nb = bank_rows(bk)[jc]
                blk = r[bk // 4, jc, bk % 4]
                if nb < NDVE:
                    o[nb * 4 : nb * 4 + 4, :] = blk.reshape(128, 4).T
                else:
                    o[nb * 4 : nb * 4 + 4, :] = blk.reshape(4, 128)
        gamma = X[b] @ V + g                          # (NX,)
        gamma[NDVE * 4 :] = 0.0
        out[b] = o + gamma[:, None] + b2[0]
    return out


def kernel(X, Y, W1, b1, W2, b2):
    in_maps = prepare_in_maps(X, Y, W1, b1, W2)
    nc = _get_nc()
    res = run_bass_kernel_spmd(nc, in_maps, core_ids=list(range(NCORES)))
    raw = [res.results[b]["out"].reshape(-1) for b in range(B)]
    return postprocess(raw, X, W1, b1, W2, b2)


if __name__ == "__main__":
    rng = np.random.default_rng(0)
    ins = {
        "X": rng.standard_normal((B, NX, D), dtype=np.float32),
        "Y": rng.standard_normal((B, NY, D), dtype=np.float32),
        "W1": rng.standard_normal((HD, 2 * D), dtype=np.float32) * (2 * D) ** -0.5,
        "b1": rng.standard_normal((HD,), dtype=np.float32) * (2 * D) ** -0.5,
        "W2": rng.standard_normal((1, HD), dtype=np.float32) * HD**-0.5,
        "b2": rng.standard_normal((1,), dtype=np.float32) * HD**-0.5,
    }
    o = kernel(**ins)
    print("kernel out:", o.shape, o.dtype, float(np.abs(o).max()))
